# revision 2
# baseline (speedup 1.0000x reference)
import numpy as np
import concourse.bass as bass
import concourse.mybir as mybir
from concourse.bass_utils import run_bass_kernel_spmd
from concourse.tile import TileContext
from concourse.mybir import AluOpType as Alu, ActivationFunctionType as Act

B, T, D, H, hd, SC, ST = 2, 2048, 1024, 16, 64, 64, 16
BT = B * T          # 4096
NC = 8
TOK = BT // NC      # 512 tokens per core
EPS = 1.1920929e-07
F32 = mybir.dt.float32
F32R = mybir.dt.float32r
BF16 = mybir.dt.bfloat16
FP8 = mybir.dt.float8e4
_DONE = object()


def _split_multi_waits(nc, max_waits=1):
    # this walrus build accepts only one sync wait per ISA instruction
    n = 0
    for f in nc.m.functions:
        for bb in f.blocks:
            out = []
            for inst in bb.instructions:
                si = inst.sync_info
                if si is not None and si.on_wait and len(si.on_wait) > max_waits:
                    for w in si.on_wait[:-max_waits]:
                        out.append(mybir.InstNoOp(
                            name=f"{inst.name}_ws{n}", ins=[], outs=[],
                            engine=inst.engine,
                            sync_info=mybir.SyncInfo(on_wait=[w], on_update=[]),
                            bass_nofuse=True))
                        n += 1
                    inst.sync_info = mybir.SyncInfo(
                        on_wait=si.on_wait[-max_waits:], on_update=si.on_update)
                out.append(inst)
            bb.instructions = out
    return n


def _zip_gens(gens, weights):
    done = [False] * len(gens)
    while not all(done):
        for i, g in enumerate(gens):
            if done[i]:
                continue
            for _ in range(weights[i]):
                if next(g, _DONE) is _DONE:
                    done[i] = True
                    break


def _build():
    nc = bass.Bass()

    xTb = nc.dram_tensor("xTb", [D, BT], BF16, kind="ExternalInput")
    xmy8 = nc.dram_tensor("xmy8", [128, 8 * TOK], F32, kind="ExternalInput")
    # interleaved weight layouts: row p of chunk k lives at [p, k*w:(k+1)*w]
    qkvw8 = nc.dram_tensor("qkvw8", [128, 8 * 384], BF16, kind="ExternalInput")
    o_w8 = nc.dram_tensor("o_w8", [128, 8 * D], BF16, kind="ExternalInput")
    in_w8 = nc.dram_tensor("in_w8", [128, 8 * SC], BF16, kind="ExternalInput")
    gate_w8 = nc.dram_tensor("gate_w8", [128, 8 * SC], BF16,
                             kind="ExternalInput")
    dt_wT = nc.dram_tensor("dt_wT", [SC, SC], BF16, kind="ExternalInput")
    BpT = nc.dram_tensor("BpT", [SC, ST], BF16, kind="ExternalInput")
    CpT = nc.dram_tensor("CpT", [SC, ST], BF16, kind="ExternalInput")
    out_wT = nc.dram_tensor("out_wT", [SC, D], BF16, kind="ExternalInput")
    negA = nc.dram_tensor("negA", [128, 8], F32, kind="ExternalInput")
    dtb = nc.dram_tensor("dtb", [SC, 1], F32, kind="ExternalInput")
    esc = nc.dram_tensor("esc", [SC, 1024], BF16, kind="ExternalInput")
    est = nc.dram_tensor("est", [ST, 128], BF16, kind="ExternalInput")
    r8 = nc.dram_tensor("r8", [128, 512], BF16, kind="ExternalInput")
    tri = nc.dram_tensor("tri", [128, 128], BF16, kind="ExternalInput")
    ident = nc.dram_tensor("ident", [128, 128], BF16, kind="ExternalInput")
    onesb = nc.dram_tensor("onesb", [1, 128], BF16, kind="ExternalInput")
    onesr = nc.dram_tensor("onesr", [1, 128], F32R, kind="ExternalInput")
    onesf = nc.dram_tensor("onesf", [128, 1], F32R, kind="ExternalInput")
    csel = nc.dram_tensor("csel", [128, 8], F32, kind="ExternalInput")
    omc = nc.dram_tensor("omc", [128, 64], F32, kind="ExternalInput")
    epsb = nc.dram_tensor("epsb", [1, 1], F32, kind="ExternalInput")

    yout = nc.dram_tensor("yout", [D, TOK], F32, kind="ExternalOutput")

    with nc.allow_low_precision(reason="bf16 compute by design"), \
         TileContext(nc) as tc:
        with tc.tile_pool(name="const", bufs=1) as cpool, \
             tc.tile_pool(name="wts", bufs=1) as wpool, \
             tc.tile_pool(name="x1", bufs=1) as x1p, \
             tc.tile_pool(name="sB", bufs=1) as spool, \
             tc.tile_pool(name="work", bufs=2) as work, \
             tc.tile_pool(name="psA", bufs=2, space="PSUM") as psA, \
             tc.tile_pool(name="psB", bufs=2, space="PSUM") as psB, \
             tc.tile_pool(name="psE", bufs=2, space="PSUM") as psE, \
             tc.tile_pool(name="psC", bufs=2, space="PSUM") as psC, \
             tc.tile_pool(name="dram", bufs=1, space="DRAM") as dram:

            # ---------------- constants (gpsimd queue) ----------------
            def csbuf(shape, src, name, d, eng=None):
                t = cpool.tile(shape, d, name=name, tag=name)
                (eng or nc.gpsimd).dma_start(t[:, :], src)
                return t

            onesfS = csbuf([128, 1], onesf[:, :], "onesfS", F32R)
            epsS = csbuf([1, 1], epsb[:, :], "epsS", F32)
            triS = csbuf([128, 128], tri[:, :], "triS", BF16)
            identS = csbuf([128, 128], ident[:, :], "identS", BF16)
            onesbS = csbuf([1, 128], onesb[:, :], "onesbS", BF16)
            onesrS = csbuf([1, 128], onesr[:, :], "onesrS", F32R)
            negAS = csbuf([128, 8], negA[:, :], "negAS", F32, nc.scalar)
            dtbS = csbuf([SC, 1], dtb[:, :], "dtbS", F32, nc.scalar)
            escS = csbuf([SC, 1024], esc[:, :], "escS", BF16, nc.scalar)
            estS = csbuf([ST, 128], est[:, :], "estS", BF16, nc.scalar)
            r8S = csbuf([128, 512], r8[:, :], "r8S", BF16, nc.scalar)
            cselS = csbuf([128, 8], csel[:, :], "cselS", F32, nc.scalar)
            omcS = csbuf([128, 64], omc[:, :], "omcS", F32, nc.scalar)

            # ---------------- residual stream (sync queue) ----------------
            x1t = x1p.tile([128, 8 * TOK], F32, name="x1t", tag="x1t")
            for q4 in range(4):
                nc.sync.dma_start(x1t[:, q4 * 2 * TOK:(q4 + 1) * 2 * TOK],
                                  xmy8[:, q4 * 2 * TOK:(q4 + 1) * 2 * TOK])
            x1 = [x1t[:, k * TOK:(k + 1) * TOK] for k in range(8)]

            # dram scratch
            cin0 = dram.tile([1, TOK], BF16, name="cin0", tag="cin0")
            cout0 = dram.tile([NC, TOK], BF16, name="cout0", tag="cout0")
            cinA = dram.tile([NC, 64, TOK], BF16, name="cinA", tag="cinA")
            coutA = dram.tile([NC, 64, TOK], BF16, name="coutA", tag="coutA")
            cinB = dram.tile([NC, 64, TOK], BF16, name="cinB", tag="cinB")
            coutB = dram.tile([NC, 64, TOK], BF16, name="coutB", tag="coutB")
            cin2 = dram.tile([128, 16], F32, name="cin2", tag="cin2")
            cout2 = dram.tile([NC * 128, 16], F32, name="cout2", tag="cout2")

            # ======================= PHASE A =======================
            with tc.tile_pool(name="pa", bufs=1) as pa:
                qkvW = pa.tile([128, 8 * 384], BF16, name="qkvW", tag="qkvW")
                nc.sync.dma_start(qkvW[:, :], qkvw8[:, :])
                xbt = [[pa.tile([128, 1024], BF16, name=f"xb{k}_{hf}",
                                tag=f"xb{k}_{hf}") for hf in range(2)]
                       for k in range(8)]
                xb = [[xbt[k][cb // 2][:, (cb % 2) * 512:(cb % 2 + 1) * 512]
                       for cb in range(4)] for k in range(8)]

                def load_xb(b, halves=(0, 1), gate=None):
                    for hf in halves:
                        for k in range(8):
                            if gate is not None:
                                nc.gpsimd.tensor_add(
                                    gate[0][:, k:k + 1],
                                    xbt[k][hf][0:1, 0:1], gate[1])
                            nc.sync.dma_start(
                                xbt[k][hf][:, :],
                                xTb[k * 128:(k + 1) * 128,
                                    b * T + hf * 1024:b * T + (hf + 1) * 1024])

                load_xb(0, halves=(0,))

                # ---- rmsnorm1 scale for my tokens ----
                sspA = psB.tile([1, TOK], F32, tag="psB")
                sspB = psE.tile([1, TOK], F32, tag="psE")
                for k in range(8):
                    sq = work.tile([128, TOK], F32R, tag="sq", bufs=4)
                    nc.vector.tensor_mul(sq[:, :], x1[k], x1[k])
                    ps_k = sspA if k % 2 == 0 else sspB
                    nc.tensor.matmul(ps_k[:, :], onesfS[:, :], sq[:, :],
                                     start=(k < 2), stop=(k >= 6))
                ssS = pa.tile([1, TOK], F32R, tag="ssS")
                nc.vector.tensor_copy(ssS[:, :], sspA[:, :])
                nc.vector.tensor_add(ssS[:, :], ssS[:, :], sspB[:, :])
                xgate = pa.tile([1, 8], F32, tag="xgate")
                load_xb(0, halves=(1,), gate=(xgate, ssS[0:1, 0:1]))
                rsv = pa.tile([1, TOK], F32, tag="rsv")
                nc.scalar.activation(rsv[:, :], ssS[:, :], Act.Sqrt,
                                     scale=1.0 / D, bias=epsS[0:1, :])
                rsvB = pa.tile([1, TOK], BF16, tag="rsvB")
                nc.vector.reciprocal(rsvB[:, :], rsv[:, :])
                nc.scalar.dma_start(cin0[:, :], rsvB[:, :])
                nc.gpsimd.collective_compute(
                    "AllGather", Alu.bypass, [list(range(NC))],
                    ins=[cin0.opt()], outs=[cout0.opt()])
                # rs for all tokens: row layout + token-partition layout
                rsF = pa.tile([NC, TOK], BF16, tag="rsF")
                nc.gpsimd.dma_start(rsF[:, :], cout0[:, :])
                rs_col = pa.tile([128, 32], F32, tag="rs_col")
                for u in range(4):
                    tp = psB.tile([128, 8], BF16, tag="psB")
                    nc.tensor.transpose(tp[:, :],
                                        rsF[0:8, 128 * u:128 * (u + 1)],
                                        identS[0:8, 0:8])
                    nc.vector.tensor_copy(rs_col[:, 8 * u:8 * u + 8],
                                          tp[:, :])
                rs125 = pa.tile([128, 32], F32, tag="rs125")
                nc.vector.tensor_scalar_mul(rs125[:, :], rs_col[:, :], 0.125)
                # row-broadcast tiles (Q scaling only)
                rsb = []
                for blk in range(8):
                    row = pa.tile([1, TOK], BF16, tag=f"rsrow{blk}")
                    nc.gpsimd.dma_start(row[:, :], cout0[blk:blk + 1, :])
                    ps = psB.tile([128, 512], F32, tag="psB")
                    nc.tensor.matmul(ps[:, :], onesbS[:, :], row[:, :],
                                     start=True, stop=True)
                    rt = pa.tile([128, 512], BF16, tag=f"rsb{blk}")
                    if blk % 2 == 0:
                        nc.scalar.copy(rt[:, :], ps[:, :])
                    else:
                        nc.vector.tensor_copy(rt[:, :], ps[:, :])
                    rsb.append(rt)

                # ---- weights: tiles now, DMAs deferred ----
                o_wP = wpool.tile([128, 8 * D], BF16, name="o_wP",
                                  tag="o_wP")
                inW = wpool.tile([128, 8 * SC], BF16, name="inW", tag="inW")
                gateW = wpool.tile([128, 8 * SC], BF16, name="gateW",
                                   tag="gateW")
                dtW = wpool.tile([SC, SC], BF16, name="dtW", tag="dtW")
                BpS = wpool.tile([SC, ST], BF16, name="BpS", tag="BpS")
                CpS = wpool.tile([SC, ST], BF16, name="CpS", tag="CpS")
                outW = wpool.tile([SC, D], BF16, name="outW", tag="outW")

                def gate_weights():
                    # force the big weight transfers to wait until the x
                    # stream is done with the DMA engines: a dummy reader of
                    # each weight tile depends on Vf[1] (end of QKV b1), and
                    # the subsequent DMA write must wait for that reader.
                    scr = pa.tile([1, 8], F32, tag="wgate")
                    for i, wt in enumerate([o_wP, inW, gateW, dtW, BpS, CpS,
                                            outW]):
                        nc.gpsimd.tensor_add(
                            scr[:, i:i + 1], wt[0:1, 0:1], Vf[1][0:1, 0:1])

                def load_scan_weights():
                    nc.gpsimd.dma_start(inW[:, :], in_w8[:, :])
                    nc.gpsimd.dma_start(gateW[:, :], gate_w8[:, :])
                    nc.gpsimd.dma_start(dtW[:, :], dt_wT[:, :])
                    nc.gpsimd.dma_start(BpS[:, :], BpT[:, :])
                    nc.gpsimd.dma_start(CpS[:, :], CpT[:, :])
                    nc.gpsimd.dma_start(outW[:, :], out_wT[:, :])

                # ---- QKV + attention ----
                Qf = [pa.tile([128, T], BF16, name=f"Qf{b}", tag=f"Qf{b}")
                      for b in range(B)]
                Kf = [pa.tile([128, T], BF16, name=f"Kf{b}", tag=f"Kf{b}")
                      for b in range(B)]
                Vf = [pa.tile([128, T], BF16, name=f"Vf{b}", tag=f"Vf{b}")
                      for b in range(B)]
                Vp = [[[pa.tile([128, 65], BF16, name=f"Vp{b}_{hh}_{kt}",
                                tag=f"Vp{b}_{hh}_{kt}") for kt in range(16)]
                       for hh in range(2)] for b in range(B)]
                for b in range(B):
                    for hh in range(2):
                        for kt in range(16):
                            nc.vector.memset(Vp[b][hh][kt][:, 64:65], 1.0)

                def qkv_units(b):
                    dsts = [Qf[b], Kf[b], Vf[b]]
                    for cb in range(4):
                        for m in range(3):
                            ps = psA.tile([128, 512], F32, tag="psA")
                            for k in range(8):
                                nc.tensor.matmul(
                                    ps[:, :],
                                    qkvW[:, k * 384 + m * 128:
                                         k * 384 + (m + 1) * 128],
                                    xb[k][cb],
                                    start=(k == 0), stop=(k == 7))
                                yield
                            dst = dsts[m][:, cb * 512:(cb + 1) * 512]
                            nc.vector.tensor_copy(dst, ps[:, :])
                            yield

                def prep_units(b):
                    # paired-head V transposes, scaled by rs at the copy
                    for kt in range(16):
                        ppool, ptag = (psB, "psB") if kt % 2 == 0 else \
                            (psE, "psE")
                        vtp = ppool.tile([128, 128], BF16, tag=ptag)
                        nc.tensor.transpose(
                            vtp[:, :], Vf[b][:, kt * 128:(kt + 1) * 128],
                            identS[:, :])
                        col = 8 * ((16 * b + kt) % 4) + (16 * b + kt) // 4
                        for hh in range(2):
                            nc.vector.tensor_scalar_mul(
                                Vp[b][hh][kt][:, 0:64],
                                vtp[:, 64 * hh:64 * hh + 64],
                                rs_col[:, col:col + 1])
                        yield
                    # in-place Q scaling (needs rs row-broadcast tiles)
                    for cb in range(4):
                        sl = Qf[b][:, cb * 512:(cb + 1) * 512]
                        nc.vector.tensor_mul(sl, sl, rsb[4 * b + cb][:, :])
                        yield

                last_stg = [None]

                def attn(b, hh, spool_ps, sptag, alt_ops=False):
                    """generator: attention for (batch b, local head hh)"""
                    cin = cinA if hh == 0 else cinB
                    r0 = 64 * hh
                    for qb in range(4):
                        q0 = qb * 512
                        if alt_ops and qb % 2 == 1:
                            ops = psA.tile([65, 512], F32, tag="psA")
                        else:
                            ops = psC.tile([65, 512], F32, tag="psC")
                        nkt = 4 * qb + 4

                        def score_mm(kt):
                            d = kt - 4 * qb
                            sp = spool_ps.tile([128, 512], F32, tag=sptag)
                            off = 128 * d if d > 0 else 0
                            nc.tensor.matmul(
                                sp[:, off:512],
                                Kf[b][r0:r0 + 64, kt * 128:(kt + 1) * 128],
                                Qf[b][r0:r0 + 64, q0 + off:q0 + 512],
                                start=True, stop=True)
                            return sp

                        sps = score_mm(0)
                        for kt in range(nkt):
                            sp = sps
                            if kt + 1 < nkt:
                                sps = score_mm(kt + 1)
                            d = kt - 4 * qb
                            off = 128 * d if d > 0 else 0
                            col = 8 * ((16 * b + kt) % 4) + (16 * b + kt) // 4
                            e = work.tile([128, 512], BF16, tag="expst",
                                          bufs=6)
                            nc.scalar.activation(e[:, off:512],
                                                 sp[:, off:512], Act.Exp,
                                                 scale=rs125[:, col:col + 1])
                            if d >= 0:
                                deng = nc.vector if b == 0 else nc.gpsimd
                                deng.tensor_mul(e[:, off:off + 128],
                                                e[:, off:off + 128],
                                                triS[:, :])
                            nc.tensor.matmul(ops[:, off:512],
                                             Vp[b][hh][kt][:, :],
                                             e[:, off:512],
                                             start=(kt == 0),
                                             stop=(kt == nkt - 1))
                            yield
                        rl = pa.tile([1, 512], F32R, tag="rl", bufs=2)
                        nc.vector.reciprocal(rl[:, :], ops[64:65, :])
                        rb = spool_ps.tile([64, 512], F32, tag=sptag)
                        nc.tensor.matmul(rb[:, :], onesrS[0:1, 0:64],
                                         rl[:, :], start=True, stop=True)
                        rbS = pa.tile([64, 512], BF16, tag="rbS", bufs=2)
                        nc.vector.tensor_copy(rbS[:, :], rb[:, :])
                        stg = pa.tile([64, 512], BF16, tag="stg", bufs=2)
                        nc.vector.tensor_mul(stg[:, :], ops[0:64, :],
                                             rbS[:, :])
                        last_stg[0] = stg
                        nc.sync.dma_start(cin[4 * b + qb, :, :], stg[:, :])
                        yield

                # ---- phase A schedule ----
                for _ in qkv_units(0):
                    pass
                load_xb(1)
                gq1 = qkv_units(1)
                gp0 = prep_units(0)
                while next(gp0, _DONE) is not _DONE:
                    next(gq1, _DONE)
                # stream both batch-0 heads, prioritizing head 0 (2x) so
                # the A2A#1 feeders finish early; QKV b1 rides along on PE
                g00 = attn(0, 0, psB, "psB")
                g01 = attn(0, 1, psE, "psE")
                done01 = False
                while True:
                    if next(g00, _DONE) is _DONE:
                        break
                    if next(g00, _DONE) is _DONE:
                        break
                    if not done01 and next(g01, _DONE) is _DONE:
                        done01 = True
                    next(gq1, _DONE)
                    next(gq1, _DONE)
                gate_weights()
                for _ in gq1:
                    pass
                for _ in prep_units(1):
                    pass
                g10 = attn(1, 0, psB, "psB", alt_ops=True)
                while True:
                    if next(g10, _DONE) is _DONE:
                        break
                    if next(g10, _DONE) is _DONE:
                        break
                    if not done01 and next(g01, _DONE) is _DONE:
                        done01 = True
                nc.gpsimd.collective_compute(
                    "AllToAll", Alu.bypass, [list(range(NC))],
                    ins=[cinA.opt()], outs=[coutA.opt()])
                g11 = attn(1, 1, psE, "psE", alt_ops=True)
                while not done01:
                    if next(g01, _DONE) is _DONE:
                        done01 = True
                    next(g11, _DONE)
                for _ in g11:
                    pass

                # o_proj A-half, accumulated straight into x1
                def oprojA_units():
                    nc.gpsimd.dma_start(o_wP[:, :], o_w8[:, :])
                    scr2 = pa.tile([1, 8], F32, tag="aggate")
                    ag = []
                    for kp in range(4):
                        t = work.tile([128, TOK], BF16, tag="ag", bufs=4)
                        # gate the coutA reads behind the end of attnB so the
                        # scheduler cannot park them at the head of the sync
                        # queue in front of the cinB staging writes
                        nc.gpsimd.tensor_add(
                            scr2[:, kp:kp + 1], t[0:1, 0:1],
                            last_stg[0][0:1, 0:1])
                        nc.sync.dma_start(
                            t[:, :],
                            coutA[2 * kp:2 * kp + 2, :, :].flatten_outer_dims())
                        ag.append(t)
                        yield
                    for m in range(8):
                        ps = psA.tile([128, 512], F32, tag="psA")
                        for kp in range(4):
                            nc.tensor.matmul(
                                ps[:, :],
                                o_wP[:, kp * D + m * 128:
                                     kp * D + (m + 1) * 128],
                                ag[kp][:, :], start=(kp == 0), stop=(kp == 3))
                            yield
                        nc.vector.tensor_add(x1[m], x1[m], ps[:, :])
                        yield

                load_scan_weights()
                nc.gpsimd.collective_compute(
                    "AllToAll", Alu.bypass, [list(range(NC))],
                    ins=[cinB.opt()], outs=[coutB.opt()])
                for _ in oprojA_units():
                    pass

            # ======================= PHASE B =======================
            agB = []
            for kp in range(4):
                t = work.tile([128, TOK], BF16, tag="ag", bufs=4)
                nc.sync.dma_start(
                    t[:, :],
                    coutB[2 * kp:2 * kp + 2, :, :].flatten_outer_dims())
                agB.append(t)
            for m in range(8):
                ps = psA.tile([128, 512], F32, tag="psA")
                for kp in range(4):
                    nc.tensor.matmul(
                        ps[:, :],
                        o_wP[:, (4 + kp) * D + m * 128:
                             (4 + kp) * D + (m + 1) * 128],
                        agB[kp][:, :], start=(kp == 0), stop=(kp == 3))
                nc.vector.tensor_add(x1[m], x1[m], ps[:, :])

            # ---- rmsnorm2 (D-major) ----
            ss2 = psB.tile([1, TOK], F32, tag="psB")
            for k in range(8):
                sq = work.tile([128, TOK], F32R, tag="sq", bufs=4)
                nc.scalar.square(sq[:, :], x1[k])
                nc.tensor.matmul(ss2[:, :], onesfS[:, :], sq[:, :],
                                 start=(k == 0), stop=(k == 7))
            rs2 = spool.tile([1, TOK], F32R, tag="rs2")
            nc.scalar.activation(rs2[:, :], ss2[:, :], Act.Sqrt,
                                 scale=1.0 / D, bias=epsS[0:1, :])
            nc.vector.reciprocal(rs2[:, :], rs2[:, :])
            rb2 = psB.tile([128, TOK], F32, tag="psB")
            nc.tensor.matmul(rb2[:, :], onesrS[:, :], rs2[:, :],
                             start=True, stop=True)
            h2 = []
            for k in range(8):
                ht = spool.tile([128, TOK], BF16, tag=f"h2_{k}")
                nc.vector.tensor_mul(ht[:, :], x1[k], rb2[:, :])
                h2.append(ht)

            # ---- scan projections ----
            pz = psC.tile([SC, TOK], F32, tag="psC")
            for k in range(8):
                nc.tensor.matmul(pz[:, :], inW[:, k * SC:(k + 1) * SC],
                                 h2[k][:, :], start=(k == 0), stop=(k == 7))
            z_s = spool.tile([SC, TOK], BF16, tag="z_s")
            nc.scalar.copy(z_s[:, :], pz[:, :])
            pg = psC.tile([SC, TOK], F32, tag="psC")
            for k in range(8):
                nc.tensor.matmul(pg[:, :], gateW[:, k * SC:(k + 1) * SC],
                                 h2[k][:, :], start=(k == 0), stop=(k == 7))
            gate_s = spool.tile([SC, TOK], BF16, tag="gate_s")
            nc.scalar.activation(gate_s[:, :], pg[:, :], Act.Silu)
            pdt = psC.tile([SC, TOK], F32, tag="psC")
            nc.tensor.matmul(pdt[:, :], dtW[:, :], z_s[:, :],
                             start=True, stop=True)
            dt_s = spool.tile([SC, TOK], BF16, tag="dt_s")
            nc.scalar.activation(dt_s[:, :], pdt[:, :], Act.Exp,
                                 bias=dtbS[:, :])
            nc.scalar.activation(dt_s[:, :], dt_s[:, :], Act.Ln, bias=1.0)
            dtz_s = spool.tile([SC, TOK], BF16, tag="dtz_s")
            nc.gpsimd.tensor_mul(dtz_s[:, :], dt_s[:, :], z_s[:, :])
            pbi = psC.tile([ST, TOK], F32, tag="psC")
            nc.tensor.matmul(pbi[:, :], BpS[:, :], z_s[:, :],
                             start=True, stop=True)
            bi_s = spool.tile([ST, TOK], BF16, tag="bi_s")
            nc.scalar.copy(bi_s[:, :], pbi[:, :])
            pci = psC.tile([ST, TOK], F32, tag="psC")
            nc.tensor.matmul(pci[:, :], CpS[:, :], z_s[:, :],
                             start=True, stop=True)
            ci_s = spool.tile([ST, TOK], BF16, tag="ci_s")
            nc.scalar.copy(ci_s[:, :], pci[:, :])
            pbe = psC.tile([128, TOK], F32, tag="psC")
            nc.tensor.matmul(pbe[:, :], estS[:, :], bi_s[:, :],
                             start=True, stop=True)
            bes = spool.tile([128, TOK], BF16, tag="bes")
            nc.scalar.copy(bes[:, :], pbe[:, :])
            pce = psC.tile([128, TOK], F32, tag="psC")
            nc.tensor.matmul(pce[:, :], estS[:, :], ci_s[:, :],
                             start=True, stop=True)
            ces = spool.tile([128, TOK], BF16, tag="ces")
            nc.scalar.copy(ces[:, :], pce[:, :])

            # ---- scan pass 1 ----
            sc0 = []
            pp = []
            tailS = spool.tile([128, 16], F32, tag="tailS")
            for g in range(8):
                pde = psA.tile([128, TOK], F32, tag="psA")
                nc.tensor.matmul(pde[:, :], escS[:, g * 128:(g + 1) * 128],
                                 dt_s[:, :], start=True, stop=True)
                abar = work.tile([128, TOK], BF16, tag="abar", bufs=2)
                nc.scalar.activation(abar[:, :], pde[:, :], Act.Copy,
                                     bias=1.0, scale=negAS[:, g:g + 1])
                pdz = psE.tile([128, TOK], F32, tag="psE")
                nc.tensor.matmul(pdz[:, :], escS[:, g * 128:(g + 1) * 128],
                                 dtz_s[:, :], start=True, stop=True)
                bin_ = work.tile([128, TOK], BF16, tag="bin_", bufs=2)
                nc.vector.tensor_mul(bin_[:, :], pdz[:, :], bes[:, :])
                s0 = spool.tile([128, TOK], BF16, tag=f"sc0_{g}")
                nc.vector.tensor_tensor_scan(s0[:, :], abar[:, :], bin_[:, :],
                                        0.0, Alu.mult, Alu.add)
                p0 = spool.tile([128, TOK], BF16, tag=f"pp_{g}")
                nc.vector.tensor_tensor_scan(p0[:, :], abar[:, :], abar[:, :],
                                        1.0, Alu.mult, Alu.bypass)
                sc0.append(s0)
                pp.append(p0)
                nc.vector.tensor_copy(tailS[:, g:g + 1], p0[:, TOK - 1:TOK])
                nc.vector.tensor_copy(tailS[:, 8 + g:8 + g + 1],
                                      s0[:, TOK - 1:TOK])

            nc.sync.dma_start(cin2[:, :], tailS[:, :])

            # local pass-2 contribution (overlaps the AllGather)
            py = psC.tile([SC, TOK], F32, tag="psC")
            for g in range(8):
                yt = work.tile([128, TOK], BF16, tag="yt", bufs=2)
                nc.gpsimd.tensor_mul(yt[:, :], sc0[g][:, :], ces[:, :])
                nc.tensor.matmul(py[:, :], r8S[:, g * 64:(g + 1) * 64],
                                 yt[:, :], start=(g == 0), stop=False)
            nc.gpsimd.collective_compute(
                "AllGather", Alu.bypass, [list(range(NC))],
                ins=[cin2.opt()], outs=[cout2.opt()])

            # stitch incoming state: one tile of all tails, then one
            # multiply-add scan over the core index per group
            pjall = spool.tile([128, 128], F32, tag="pjall")
            nc.scalar.dma_start(
                pjall[:, :].rearrange("p (j c) -> p j c", c=16),
                cout2[:, :].rearrange("(j p) c -> p j c", p=128))
            # pe_all[:, 8g+j] = pp_tail[j,g]*sel[j] + (1-sel[j])
            # se_all[:, 8g+j] = sc0_tail[j,g]*sel[j]
            pe_all = spool.tile([128, 64], F32, tag="pe_all")
            se_all = spool.tile([128, 64], F32, tag="se_all")
            for g in range(8):
                nc.vector.scalar_tensor_tensor(
                    pe_all[:, 8 * g:8 * g + 8], pjall[:, g:128:16],
                    1.0, cselS[:, 0:8], Alu.mult, Alu.mult)
                nc.vector.tensor_add(pe_all[:, 8 * g:8 * g + 8],
                                     pe_all[:, 8 * g:8 * g + 8],
                                     omcS[:, 0:64:8])
                nc.vector.tensor_mul(se_all[:, 8 * g:8 * g + 8],
                                     pjall[:, 8 + g:128:16], cselS[:, 0:8])
            sinx = spool.tile([128, 64], F32, tag="sinx")
            for g in range(8):
                nc.vector.tensor_tensor_scan(
                    sinx[:, 8 * g:8 * g + 8], pe_all[:, 8 * g:8 * g + 8],
                    se_all[:, 8 * g:8 * g + 8], 0.0, Alu.mult, Alu.add)

            # correction terms
            for g in range(8):
                cpp = work.tile([128, TOK], BF16, tag="cpp", bufs=2)
                nc.vector.tensor_scalar_mul(cpp[:, :], pp[g][:, :],
                                            sinx[:, 8 * g + 7:8 * g + 8])
                ytc = work.tile([128, TOK], BF16, tag="ytc", bufs=2)
                nc.vector.tensor_mul(ytc[:, :], cpp[:, :], ces[:, :])
                nc.tensor.matmul(py[:, :], r8S[:, g * 64:(g + 1) * 64],
                                 ytc[:, :], start=False, stop=(g == 7))

            # gate + out_proj + final residual
            yg = spool.tile([SC, TOK], BF16, tag="yg")
            nc.vector.tensor_mul(yg[:, :], py[:, :], gate_s[:, :])
            for m in range(8):
                p2 = psC.tile([128, TOK], F32, tag="psC")
                nc.tensor.matmul(p2[:, :], outW[:, m * 128:(m + 1) * 128],
                                 yg[:, :], start=True, stop=True)
                nc.vector.tensor_add(x1[m], x1[m], p2[:, :])
                nc.sync.dma_start(yout[m * 128:(m + 1) * 128, :], x1[m])

    _split_multi_waits(nc)
    return nc


def _interleave_rows(w, nk, width):
    # w: [nk*128, width] -> [128, nk*width] with chunk k at cols k*width
    out = np.empty((128, nk * width), w.dtype)
    for k in range(nk):
        out[:, k * width:(k + 1) * width] = w[k * 128:(k + 1) * 128, :]
    return np.ascontiguousarray(out)


def kernel(x, qkv_w, o_w, norm1_w, norm2_w, in_w, out_w, A_log, Bp_w, Cp_w,
           dt_w, dt_b, gate_w):
    import ml_dtypes
    f = np.float32
    bf = ml_dtypes.bfloat16
    xf = np.asarray(x, f).reshape(BT, D)
    xT = np.ascontiguousarray(xf.T)
    xTb = np.ascontiguousarray(xT.astype(bf))
    w1 = np.asarray(norm1_w, f)
    w2 = np.asarray(norm2_w, f)
    qkv_w1 = np.asarray(qkv_w, f) * w1[None, :]

    # o_w.T with packed row order: A-half rows = heads (4k',4k'+2), B-half
    # rows = heads (4k'+1,4k'+3), k'=0..3
    o_wT = np.asarray(o_w, f).T  # [attn_dim, m]
    rows = []
    for half in range(2):
        for kp in range(4):
            for h in (4 * kp + half, 4 * kp + 2 + half):
                rows.extend(range(64 * h, 64 * h + 64))
    o_w8 = _interleave_rows(o_wT[rows, :].astype(bf), 8, D)

    in_w8 = _interleave_rows(
        (np.asarray(in_w, f) * w2[None, :]).T.astype(bf), 8, SC)
    gate_w8 = _interleave_rows(
        (np.asarray(gate_w, f) * w2[None, :]).T.astype(bf), 8, SC)
    dt_wT = np.ascontiguousarray(np.asarray(dt_w, f).T.astype(bf))
    BpT = np.ascontiguousarray(np.asarray(Bp_w, f).T.astype(bf))
    CpT = np.ascontiguousarray(np.asarray(Cp_w, f).T.astype(bf))
    out_wT = np.ascontiguousarray(np.asarray(out_w, f).T.astype(bf))

    A_log = np.asarray(A_log, f)
    negAm = np.empty((128, 8), f)
    for g in range(8):
        for p in range(128):
            negAm[p, g] = -np.exp(A_log[8 * g + p // 16, p % 16])
    dtbv = np.ascontiguousarray(np.asarray(dt_b, f).reshape(SC, 1))
    jj = np.arange(1024)
    escm = (np.arange(SC)[:, None] == (jj[None, :] // 16)).astype(bf)
    estm = (np.arange(ST)[:, None] == (np.arange(128)[None, :] % 16)).astype(bf)
    r8m = np.zeros((128, 512), f)
    for g in range(8):
        for j in range(128):
            r8m[j, g * 64 + 8 * g + j // 16] = 1.0
    r8m = r8m.astype(bf)
    tri_m = (np.arange(128)[None, :] >= np.arange(128)[:, None]).astype(bf)
    ident = np.eye(128, dtype=np.float32).astype(bf)

    nc = _build()
    in_maps = []
    for c in range(NC):
        b, q = c // 4, c % 4
        h0 = 2 * c
        rows_q = np.concatenate([np.arange(h0 * 64, (h0 + 2) * 64),
                                 D + np.arange(h0 * 64, (h0 + 2) * 64),
                                 2 * D + np.arange(h0 * 64, (h0 + 2) * 64)])
        qkvw8 = _interleave_rows(qkv_w1[rows_q, :].T.astype(bf), 8, 384)
        sel = np.zeros(NC, f)
        for j in range(q):
            sel[4 * b + j] = 1.0
        cselv = np.ascontiguousarray(np.tile(sel[None, :], (128, 1)))
        omcv = np.ascontiguousarray(
            np.repeat(1.0 - sel, 8)[None, :].repeat(128, axis=0).astype(f))
        in_maps.append({
            "xTb": xTb,
            "xmy8": _interleave_rows(
                np.ascontiguousarray(xT[:, c * TOK:(c + 1) * TOK]), 8, TOK),
            "qkvw8": qkvw8, "o_w8": o_w8, "in_w8": in_w8,
            "gate_w8": gate_w8, "dt_wT": dt_wT, "BpT": BpT, "CpT": CpT,
            "out_wT": out_wT, "negA": negAm, "dtb": dtbv, "esc": escm,
            "est": estm, "r8": r8m, "tri": tri_m, "ident": ident,
            "onesb": np.ones((1, 128), bf), "onesr": np.ones((1, 128), f),
            "onesf": np.ones((128, 1), f), "csel": cselv, "omc": omcv,
            "epsb": np.full((1, 1), EPS, f),
        })
    res = run_bass_kernel_spmd(nc, in_maps, core_ids=list(range(NC)))
    out = np.stack([np.asarray(res.results[c]["yout"], np.float32).T
                    for c in range(NC)], axis=0)
    return out.reshape(B, T, D).astype(np.float32)
